# revision 23
# baseline (speedup 1.0000x reference)
"""Trainium2 Bass kernel for nn_AttentionPositionAlign.

Reference computation (per batch b):
    src = query @ Wq                    # [M, H]
    tgt = memory @ Wm                   # [N, H]
    aligns = relu(src[:,None,:] + tgt[None,:,:])   # [M, N, H]
    out = aligns.reshape(M, N*H) @ Wout # [M, 4]

Strategy: data-parallel over B across the 8 NeuronCores (B == 8). All
compute happens in "transposed land" (H on SBUF partitions, M on the free
dim) so the Bahdanau broadcast-add becomes a per-partition scalar bias
that fuses into a single elementwise pass — the [B,M,N,H] intermediate
(604 MB) is never materialized:

    srcT[h, m] = (Wq.T @ query.T)[h, m]         PSUM-accumulated matmuls
    tgt_nh[n, h] = (memory @ Wm)[n, h]          (+ PE transposes to [h, n])
    for each (hc, n) chunk c (N*H/128 = 144 of them):
        Rt = relu(srcT[hc] + tgtT[hc][:, n])    ONE fused op per chunk:
                                                DVE tensor_scalar(add,max)
                                                or ACT activation(Relu,bias),
                                                split ~70/30 by throughput
        psum_out[32g+k, m] += Wout_c.T @ Rt     col-tiled (tile_position)
                                                matmuls, 4 concurrent PE
                                                column groups, 144-deep
                                                PSUM accumulation
    out[k, m] = sum_g psum_out[32g+k, m]        selector matmul, then host
                                                transposes [4, M] -> [M, 4]

Inputs ship bf16 (halves the DMA-bound head); the relu path and Wout
contraction run bf16 (DVE 4x mode); the cross-partition reduce and the
selector run float32r (fp32 bits, full-speed PE). Measured ~4e-3 max
relative error. Set IN_DT=R_DT=SRC_DT=f32r for ~2-3e-4 at ~15% more time.
"""

import numpy as np

import concourse.bass as bass
import concourse.tile as tile
from concourse import bacc, mybir
from concourse.bass_utils import run_bass_kernel_spmd

B, M, N, H = 8, 1024, 36, 512
DQ, DM = 512, 2048
P = 128
HC = H // P          # 4 h-chunks
DQC = DQ // P        # 4
DMC = DM // P        # 16
MC = 2               # m-chunks for 512-wide PSUM banks
MF = M // MC         # 512
NCHUNK = N * HC      # 144 contraction chunks of 128

f32 = mybir.dt.float32
f32r = mybir.dt.float32r
bf16 = mybir.dt.bfloat16

# Knobs
R_DT = bf16          # dtype of the relu output / contraction rhs+lhsT
SRC_DT = bf16        # dtype of the srcT store / relu input
IN_DT = bf16         # dtype inputs are shipped in (f32r or bf16)
# per-chunk relu engine assignment: v=DVE, a=ACT, g=GpSimd, weighted by
# measured per-op rates (DVE ~580ns, GpSimd ~850ns?, ACT ~1360ns)
ENGINE_PATTERN = "vvavavavvv"  # 7v 3a per 10 (GpSimd TS is ~15us/op - unusable)
COL_TILE = 4         # concurrent PE column groups for the contraction (1 or 4)

_CACHE = {}


def _assign_engine(c):
    return ENGINE_PATTERN[c % len(ENGINE_PATTERN)]


def _build():
    nc = bacc.Bacc("TRN2", target_bir_lowering=False, debug=False, num_devices=B)

    qT = nc.dram_tensor("qT", [P, DQC * M], IN_DT, kind="ExternalInput").ap()
    sel = (
        nc.dram_tensor("sel", [P, 4], f32r, kind="ExternalInput").ap()
        if COL_TILE > 1
        else None
    )
    mT = nc.dram_tensor("mT", [P, DMC * N], IN_DT, kind="ExternalInput").ap()
    wq = nc.dram_tensor("wq", [P, DQC * H], IN_DT, kind="ExternalInput").ap()
    wm = nc.dram_tensor("wm", [P, DMC * H], IN_DT, kind="ExternalInput").ap()
    wo = nc.dram_tensor("wo", [P, NCHUNK * 4], R_DT, kind="ExternalInput").ap()
    out = nc.dram_tensor("out", [4, M], f32, kind="ExternalOutput").ap()

    with tile.TileContext(nc) as tc:
        with (
            tc.tile_pool(name="weights", bufs=1) as wpool,
            tc.tile_pool(name="acts", bufs=1) as apool,
            tc.tile_pool(name="rpool", bufs=12) as rpool,
            tc.tile_pool(name="ppool", bufs=2, space="PSUM") as ppool,
            tc.tile_pool(name="opool", bufs=1, space="PSUM") as opool,
        ):
            # --- load inputs, chunked so compute pipelines with DMA.
            # wm is packed (hc, dm)-major so tgt[hc] unblocks after 1MB.
            mT_sb = wpool.tile([P, DMC, N], IN_DT)
            nc.sync.dma_start(mT_sb[:], mT[:])
            wm_sb = wpool.tile([P, HC, DMC, P], IN_DT)

            def load_wm(hc):
                for dmh in range(2):
                    lo = hc * DMC * P + dmh * (DMC // 2) * P
                    nc.sync.dma_start(
                        wm_sb[:, hc, dmh * (DMC // 2) : (dmh + 1) * (DMC // 2), :],
                        wm[:, lo : lo + (DMC // 2) * P],
                    )

            # wm[hc0] first: the first relu chunk is gated on tgt[hc0], which
            # needs only mT + this 0.5MB slice.
            load_wm(0)
            wq_sb = wpool.tile([P, DQC, H], IN_DT)
            qT_sb = wpool.tile([P, DQC, M], IN_DT)
            for dq in range(DQC):
                nc.sync.dma_start(wq_sb[:, dq, :], wq[:, dq * H : (dq + 1) * H])
                nc.sync.dma_start(qT_sb[:, dq, :], qT[:, dq * M : (dq + 1) * M])
            wo_sb = wpool.tile([P, NCHUNK * 4], R_DT)
            nc.sync.dma_start(wo_sb[:], wo[:])
            for hc in range(1, HC):
                load_wm(hc)

            # --- fully interleaved per-hc pipeline: src proj, tgt proj,
            # then that hc's 36 relu+contraction chunks. Emission order
            # matches dataflow readiness so the per-engine semaphore
            # thresholds don't serialize the relu stage behind later
            # projections.
            ident = wpool.tile([P, P], f32)
            from concourse.masks import make_identity

            make_identity(nc, ident[:])
            assert COL_TILE > 1
            po = [opool.tile([P, MF], f32, name=f"po{mc}") for mc in range(MC)]
            # zero-fill all 128 partitions (sets has_written) so the
            # col-group matmuls can accumulate and the final full-width
            # read sees no stale PSUM garbage
            zw = wpool.tile([P, MF], R_DT)
            nc.any.memset(zw[:], 0.0)
            for mc in range(MC):
                nc.tensor.matmul(
                    po[mc][:], zw[:, :P], zw[:],
                    start=True, stop=False, skip_group_check=True,
                )

            srcT_sb = apool.tile([P, HC, M], SRC_DT)
            tgt_sb = apool.tile([P, HC, N], f32)
            for hc in range(HC):
                # srcT[hc]
                for mc in range(MC):
                    ps = ppool.tile([P, MF], f32, tag="proj")
                    for dq in range(DQC):
                        nc.tensor.matmul(
                            ps[:],
                            wq_sb[:, dq, hc * P : (hc + 1) * P],
                            qT_sb[:, dq, mc * MF : (mc + 1) * MF],
                            start=(dq == 0),
                            stop=(dq == DQC - 1),
                        )
                    nc.scalar.copy(srcT_sb[:, hc, mc * MF : (mc + 1) * MF], ps[:])
                # tgt[hc] (swapped matmul + PE transpose)
                pt = ppool.tile([N, P], f32, tag="tproj")
                for dm in range(DMC):
                    nc.tensor.matmul(
                        pt[:],
                        mT_sb[:, dm, :],
                        wm_sb[:, hc, dm, :],
                        start=(dm == 0),
                        stop=(dm == DMC - 1),
                    )
                tgt_nh = apool.tile([N, P], f32, tag="tgtnh_sb")
                nc.scalar.copy(tgt_nh[:], pt[:])
                pz = ppool.tile([P, N], f32, tag="tproj")
                nc.tensor.transpose(pz[:], tgt_nh[:], ident[:N, :N])
                nc.scalar.copy(tgt_sb[:, hc, :], pz[:])
                # this hc's relu + contraction chunks
                for n in range(N):
                    c = hc * N + n
                    r = rpool.tile([P, M], R_DT)
                    bias = tgt_sb[:, hc, n : n + 1]
                    if _assign_engine(c) == "a":
                        nc.scalar.activation(
                            r[:],
                            srcT_sb[:, hc, :],
                            mybir.ActivationFunctionType.Relu,
                            bias=bias,
                            scale=1.0,
                        )
                    else:
                        nc.vector.tensor_scalar(
                            r[:],
                            srcT_sb[:, hc, :],
                            bias,
                            0.0,
                            mybir.AluOpType.add,
                            mybir.AluOpType.max,
                        )
                    g = c % COL_TILE
                    for mc in range(MC):
                        nc.tensor.matmul(
                            po[mc][32 * g : 32 * g + 4, :],
                            wo_sb[:, 4 * c : 4 * c + 4],
                            r[:, mc * MF : (mc + 1) * MF],
                            start=False,
                            stop=(c >= NCHUNK - COL_TILE),
                            tile_position=(0, 32 * g),
                            skip_group_check=True,
                        )

            # --- write out ---
            out_sb = apool.tile([4, M], f32)
            if COL_TILE > 1:
                # cross-partition reduce of the 4 column groups via a
                # selector matmul: out[k, m] = sum_g po[32g+k, m]
                sel_sb = wpool.tile([P, 4], f32r)
                nc.sync.dma_start(sel_sb[:], sel[:])
                for mc in range(MC):
                    pf = apool.tile([P, MF], f32r, name=f"pf{mc}")
                    nc.vector.tensor_copy(pf[:], po[mc][:])
                    ro = opool.tile([4, MF], f32, tag="redout")
                    nc.tensor.matmul(ro[:], sel_sb[:], pf[:], start=True, stop=True)
                    nc.scalar.copy(out_sb[:, mc * MF : (mc + 1) * MF], ro[:])
            else:
                for mc in range(MC):
                    nc.scalar.copy(out_sb[:, mc * MF : (mc + 1) * MF], po[mc][:])
            nc.sync.dma_start(out[:], out_sb[:])

    nc.compile()
    return nc


def _sel_array():
    s = np.zeros((P, 4), np.float32)
    for p in range(P):
        if p % 32 < 4:
            s[p, p % 32] = 1.0
    return s


def _np_in_dt():
    if IN_DT == bf16:
        import ml_dtypes

        return ml_dtypes.bfloat16
    return np.float32


def _pack_partition_major(a, chunks):
    """[chunks*128, X] -> [128, chunks*X] with chunk-major free dim."""
    x = a.shape[1]
    return (
        np.ascontiguousarray(a.reshape(chunks, P, x).transpose(1, 0, 2))
        .reshape(P, chunks * x)
        .astype(_np_in_dt())
    )


def kernel(query, memory, Wq, Wm, Wout):
    if "nc" not in _CACHE:
        _CACHE["nc"] = _build()
    nc = _CACHE["nc"]
    in_maps = _make_in_maps(query, memory, Wq, Wm, Wout)
    res = run_bass_kernel_spmd(nc, in_maps, list(range(B)))
    return np.stack([res.results[b]["out"].T for b in range(B)]).astype(np.float32)


def _make_in_maps(query, memory, Wq, Wm, Wout):
    wq_p = _pack_partition_major(np.asarray(Wq, np.float32), DQC)
    # wm packed [hi, (hc, dm, 128)]: Wm[dm*128+hi, hc*128+hin]
    wm_p = (
        np.ascontiguousarray(
            np.asarray(Wm, np.float32).reshape(DMC, P, HC, P).transpose(1, 2, 0, 3)
        )
        .reshape(P, DM * HC * P // P)
        .astype(_np_in_dt())
    )
    # Wout rows are n*H + hc*128 + p; kernel chunk id c = hc*N + n (hc-major)
    wo_p = np.ascontiguousarray(
        np.asarray(Wout, np.float32).reshape(N, HC, P, 4).transpose(2, 1, 0, 3)
    ).reshape(P, NCHUNK * 4)
    if R_DT == bf16:
        import ml_dtypes

        wo_p = wo_p.astype(ml_dtypes.bfloat16)
    in_maps = []
    for b in range(B):
        qT_p = _pack_partition_major(
            np.ascontiguousarray(np.asarray(query[b], np.float32).T), DQC
        )
        mT_p = _pack_partition_major(
            np.ascontiguousarray(np.asarray(memory[b], np.float32).T), DMC
        )
        m = {"qT": qT_p, "mT": mT_p, "wq": wq_p, "wm": wm_p, "wo": wo_p}
        if COL_TILE > 1:
            m["sel"] = _sel_array()
        in_maps.append(m)
    return in_maps


def bench(inputs, iters=20):
    """Time repeated executions of the compiled kernel with inputs resident
    on device. Returns a list of per-call wall seconds."""
    import time

    import jax
    from jax.sharding import Mesh, PartitionSpec
    from jax.experimental.shard_map import shard_map

    from concourse import bass2jax, mybir as _mybir

    if "nc" not in _CACHE:
        _CACHE["nc"] = _build()
    nc = _CACHE["nc"]
    in_maps = _make_in_maps(**inputs)

    bass2jax.install_neuronx_cc_hook()
    partition_name = nc.partition_id_tensor.name if nc.partition_id_tensor else None
    in_names, out_names, out_avals, zero_outs = [], [], [], []
    for alloc in nc.m.functions[0].allocations:
        if not isinstance(alloc, _mybir.MemoryLocationSet):
            continue
        name = alloc.memorylocations[0].name
        if alloc.kind == "ExternalInput":
            if name != partition_name:
                in_names.append(name)
        elif alloc.kind == "ExternalOutput":
            shape = tuple(alloc.tensor_shape)
            dtype = _mybir.dt.np(alloc.dtype)
            out_names.append(name)
            out_avals.append(jax.core.ShapedArray(shape, dtype))
            zero_outs.append(np.zeros(shape, dtype))
    n_params = len(in_names)
    n_outs = len(out_avals)
    all_in_names = list(in_names) + list(out_names)
    if partition_name is not None:
        all_in_names.append(partition_name)

    def _body(*args):
        operands = list(args)
        if partition_name is not None:
            operands.append(bass2jax.partition_id_tensor())
        outs = bass2jax._bass_exec_p.bind(
            *operands,
            out_avals=tuple(out_avals),
            in_names=tuple(all_in_names),
            out_names=tuple(out_names),
            lowering_input_output_aliases=(),
            sim_require_finite=True,
            sim_require_nnan=True,
            nc=nc,
        )
        return tuple(outs)

    devices = jax.devices()[:B]
    mesh = Mesh(np.asarray(devices), ("core",))
    in_specs = (PartitionSpec("core"),) * (n_params + n_outs)
    out_specs = (PartitionSpec("core"),) * n_outs
    sharded = jax.jit(
        shard_map(
            _body, mesh=mesh, in_specs=in_specs, out_specs=out_specs, check_rep=False
        ),
        donate_argnums=tuple(range(n_params, n_params + n_outs)),
        keep_unused=True,
    )
    concat_in = [
        np.concatenate([np.asarray(in_maps[c][nm]) for c in range(B)], axis=0)
        for nm in in_names
    ]
    dev_in = [jax.device_put(a) for a in concat_in]

    def zeros():
        return [np.zeros((B * z.shape[0], *z.shape[1:]), z.dtype) for z in zero_outs]

    # warmup (compile)
    out = sharded(*dev_in, *zeros())
    jax.block_until_ready(out)

    times = []
    for _ in range(iters):
        t0 = time.perf_counter()
        out = sharded(*dev_in, *zeros())
        jax.block_until_ready(out)
        times.append(time.perf_counter() - t0)
    return times


# revision 24
# speedup vs baseline: 1.0402x; 1.0402x over previous
"""Trainium2 Bass kernel for nn_AttentionPositionAlign.

Reference computation (per batch b):
    src = query @ Wq                    # [M, H]
    tgt = memory @ Wm                   # [N, H]
    aligns = relu(src[:,None,:] + tgt[None,:,:])   # [M, N, H]
    out = aligns.reshape(M, N*H) @ Wout # [M, 4]

Strategy: data-parallel over B across the 8 NeuronCores (B == 8). All
compute happens in "transposed land" (H on SBUF partitions, M on the free
dim) so the Bahdanau broadcast-add becomes a per-partition scalar bias
that fuses into a single elementwise pass — the [B,M,N,H] intermediate
(604 MB) is never materialized:

    srcT[h, m] = (Wq.T @ query.T)[h, m]         PSUM-accumulated matmuls
    tgt_nh[n, h] = (memory @ Wm)[n, h]          (+ PE transposes to [h, n])
    for each (hc, n) chunk c (N*H/128 = 144 of them):
        Rt = relu(srcT[hc] + tgtT[hc][:, n])    ONE fused op per chunk:
                                                DVE tensor_scalar(add,max)
                                                or ACT activation(Relu,bias),
                                                split ~70/30 by throughput
        psum_out[32g+k, m] += Wout_c.T @ Rt     col-tiled (tile_position)
                                                matmuls, 4 concurrent PE
                                                column groups, 144-deep
                                                PSUM accumulation
    out[k, m] = sum_g psum_out[32g+k, m]        selector matmul, then host
                                                transposes [4, M] -> [M, 4]

Inputs ship bf16 (halves the DMA-bound head); the relu path and Wout
contraction run bf16 (DVE 4x mode); the cross-partition reduce and the
selector run float32r (fp32 bits, full-speed PE). Measured ~4e-3 max
relative error. Set IN_DT=R_DT=SRC_DT=f32r for ~2-3e-4 at ~15% more time.
"""

import numpy as np

import concourse.bass as bass
import concourse.tile as tile
from concourse import bacc, mybir
from concourse.bass_utils import run_bass_kernel_spmd

B, M, N, H = 8, 1024, 36, 512
DQ, DM = 512, 2048
P = 128
HC = H // P          # 4 h-chunks
DQC = DQ // P        # 4
DMC = DM // P        # 16
MC = 2               # m-chunks for 512-wide PSUM banks
MF = M // MC         # 512
NCHUNK = N * HC      # 144 contraction chunks of 128

f32 = mybir.dt.float32
f32r = mybir.dt.float32r
bf16 = mybir.dt.bfloat16

# Knobs
R_DT = bf16          # dtype of the relu output / contraction rhs+lhsT
SRC_DT = bf16        # dtype of the srcT store / relu input
IN_DT = bf16         # dtype inputs are shipped in (f32r or bf16)
# per-chunk relu engine assignment: v=DVE, a=ACT, g=GpSimd, weighted by
# measured per-op rates (DVE ~580ns, GpSimd ~850ns?, ACT ~1360ns)
ENGINE_PATTERN = "vvavavavvv"  # 7v 3a per 10 (GpSimd TS is ~15us/op - unusable)
COL_TILE = 4         # concurrent PE column groups for the contraction (1 or 4)

_CACHE = {}


def _assign_engine(c):
    return ENGINE_PATTERN[c % len(ENGINE_PATTERN)]


def _build():
    nc = bacc.Bacc("TRN2", target_bir_lowering=False, debug=False, num_devices=B)

    qT = nc.dram_tensor("qT", [P, DQC * M], IN_DT, kind="ExternalInput").ap()
    sel = (
        nc.dram_tensor("sel", [P, 4], f32r, kind="ExternalInput").ap()
        if COL_TILE > 1
        else None
    )
    mT = nc.dram_tensor("mT", [P, DMC * N], IN_DT, kind="ExternalInput").ap()
    wq = nc.dram_tensor("wq", [P, DQC * H], IN_DT, kind="ExternalInput").ap()
    wm = nc.dram_tensor("wm", [P, DMC * H], IN_DT, kind="ExternalInput").ap()
    wo = nc.dram_tensor("wo", [P, NCHUNK * 4], R_DT, kind="ExternalInput").ap()
    out = nc.dram_tensor("out", [4, M], f32, kind="ExternalOutput").ap()

    with tile.TileContext(nc) as tc:
        with (
            tc.tile_pool(name="weights", bufs=1) as wpool,
            tc.tile_pool(name="acts", bufs=1) as apool,
            tc.tile_pool(name="rpool", bufs=12) as rpool,
            tc.tile_pool(name="ppool", bufs=2, space="PSUM") as ppool,
            tc.tile_pool(name="opool", bufs=1, space="PSUM") as opool,
        ):
            # --- load inputs, chunked so compute pipelines with DMA.
            # wm is packed (hc, dm)-major so tgt[hc] unblocks after 1MB.
            mT_sb = wpool.tile([P, DMC, N], IN_DT)
            nc.sync.dma_start(mT_sb[:], mT[:])
            wm_sb = wpool.tile([P, HC, DMC, P], IN_DT)

            def load_wm(hc):
                for dmh in range(2):
                    lo = hc * DMC * P + dmh * (DMC // 2) * P
                    nc.sync.dma_start(
                        wm_sb[:, hc, dmh * (DMC // 2) : (dmh + 1) * (DMC // 2), :],
                        wm[:, lo : lo + (DMC // 2) * P],
                    )

            # wm[hc0] first: the first relu chunk is gated on tgt[hc0], which
            # needs only mT + this 0.5MB slice.
            load_wm(0)
            wq_sb = wpool.tile([P, DQC, H], IN_DT)
            qT_sb = wpool.tile([P, DQC, M], IN_DT)
            for dq in range(DQC):
                nc.sync.dma_start(wq_sb[:, dq, :], wq[:, dq * H : (dq + 1) * H])
                nc.sync.dma_start(qT_sb[:, dq, :], qT[:, dq * M : (dq + 1) * M])
            wo_sb = wpool.tile([P, NCHUNK * 4], R_DT)
            nc.sync.dma_start(wo_sb[:], wo[:])
            for hc in range(1, HC):
                load_wm(hc)

            # --- fully interleaved per-hc pipeline: src proj, tgt proj,
            # then that hc's 36 relu+contraction chunks. Emission order
            # matches dataflow readiness so the per-engine semaphore
            # thresholds don't serialize the relu stage behind later
            # projections.
            ident = wpool.tile([P, P], f32)
            from concourse.masks import make_identity

            make_identity(nc, ident[:])
            assert COL_TILE > 1
            po = [opool.tile([P, MF], f32, name=f"po{mc}") for mc in range(MC)]
            # zero-fill all 128 partitions (sets has_written) so the
            # col-group matmuls can accumulate and the final full-width
            # read sees no stale PSUM garbage
            zw = wpool.tile([P, MF], R_DT)
            nc.any.memset(zw[:], 0.0)
            for mc in range(MC):
                nc.tensor.matmul(
                    po[mc][:], zw[:, :P], zw[:],
                    start=True, stop=False, skip_group_check=True,
                )

            srcT_sb = apool.tile([P, HC, M], SRC_DT)
            tgt_sb = apool.tile([P, HC, N], f32)

            def proj(hc):
                # srcT[hc]
                for mc in range(MC):
                    ps = ppool.tile([P, MF], f32, tag="proj")
                    for dq in range(DQC):
                        nc.tensor.matmul(
                            ps[:],
                            wq_sb[:, dq, hc * P : (hc + 1) * P],
                            qT_sb[:, dq, mc * MF : (mc + 1) * MF],
                            start=(dq == 0),
                            stop=(dq == DQC - 1),
                        )
                    nc.scalar.copy(srcT_sb[:, hc, mc * MF : (mc + 1) * MF], ps[:])
                # tgt[hc] (swapped matmul + PE transpose)
                pt = ppool.tile([N, P], f32, tag="tproj")
                for dm in range(DMC):
                    nc.tensor.matmul(
                        pt[:],
                        mT_sb[:, dm, :],
                        wm_sb[:, hc, dm, :],
                        start=(dm == 0),
                        stop=(dm == DMC - 1),
                    )
                tgt_nh = apool.tile([N, P], f32, tag="tgtnh_sb")
                nc.scalar.copy(tgt_nh[:], pt[:])
                pz = ppool.tile([P, N], f32, tag="tproj")
                nc.tensor.transpose(pz[:], tgt_nh[:], ident[:N, :N])
                nc.scalar.copy(tgt_sb[:, hc, :], pz[:])

            def chunks(hc):
                # this hc's relu + contraction chunks
                for n in range(N):
                    c = hc * N + n
                    r = rpool.tile([P, M], R_DT)
                    bias = tgt_sb[:, hc, n : n + 1]
                    if _assign_engine(c) == "a":
                        nc.scalar.activation(
                            r[:],
                            srcT_sb[:, hc, :],
                            mybir.ActivationFunctionType.Relu,
                            bias=bias,
                            scale=1.0,
                        )
                    else:
                        nc.vector.tensor_scalar(
                            r[:],
                            srcT_sb[:, hc, :],
                            bias,
                            0.0,
                            mybir.AluOpType.add,
                            mybir.AluOpType.max,
                        )
                    g = c % COL_TILE
                    for mc in range(MC):
                        nc.tensor.matmul(
                            po[mc][32 * g : 32 * g + 4, :],
                            wo_sb[:, 4 * c : 4 * c + 4],
                            r[:, mc * MF : (mc + 1) * MF],
                            start=False,
                            stop=(c >= NCHUNK - COL_TILE),
                            tile_position=(0, 32 * g),
                            skip_group_check=True,
                        )

            # Software-pipelined emission: hc+1's projections go into the PE
            # stream BEFORE hc's 72 contraction matmuls, so srcT/tgt of the
            # next hc are ready when the producers reach the boundary
            # (in-order PE queue would otherwise stall DVE/ACT 2-5us per hc).
            proj(0)
            for hc in range(HC):
                if hc + 1 < HC:
                    proj(hc + 1)
                chunks(hc)

            # --- write out ---
            out_sb = apool.tile([4, M], f32)
            if COL_TILE > 1:
                # cross-partition reduce of the 4 column groups via a
                # selector matmul: out[k, m] = sum_g po[32g+k, m]
                sel_sb = wpool.tile([P, 4], f32r)
                nc.sync.dma_start(sel_sb[:], sel[:])
                for mc in range(MC):
                    pf = apool.tile([P, MF], f32r, name=f"pf{mc}")
                    nc.vector.tensor_copy(pf[:], po[mc][:])
                    ro = opool.tile([4, MF], f32, tag="redout")
                    nc.tensor.matmul(ro[:], sel_sb[:], pf[:], start=True, stop=True)
                    nc.scalar.copy(out_sb[:, mc * MF : (mc + 1) * MF], ro[:])
            else:
                for mc in range(MC):
                    nc.scalar.copy(out_sb[:, mc * MF : (mc + 1) * MF], po[mc][:])
            nc.sync.dma_start(out[:], out_sb[:])

    nc.compile()
    return nc


def _sel_array():
    s = np.zeros((P, 4), np.float32)
    for p in range(P):
        if p % 32 < 4:
            s[p, p % 32] = 1.0
    return s


def _np_in_dt():
    if IN_DT == bf16:
        import ml_dtypes

        return ml_dtypes.bfloat16
    return np.float32


def _pack_partition_major(a, chunks):
    """[chunks*128, X] -> [128, chunks*X] with chunk-major free dim."""
    x = a.shape[1]
    return (
        np.ascontiguousarray(a.reshape(chunks, P, x).transpose(1, 0, 2))
        .reshape(P, chunks * x)
        .astype(_np_in_dt())
    )


def kernel(query, memory, Wq, Wm, Wout):
    if "nc" not in _CACHE:
        _CACHE["nc"] = _build()
    nc = _CACHE["nc"]
    in_maps = _make_in_maps(query, memory, Wq, Wm, Wout)
    res = run_bass_kernel_spmd(nc, in_maps, list(range(B)))
    return np.stack([res.results[b]["out"].T for b in range(B)]).astype(np.float32)


def _make_in_maps(query, memory, Wq, Wm, Wout):
    wq_p = _pack_partition_major(np.asarray(Wq, np.float32), DQC)
    # wm packed [hi, (hc, dm, 128)]: Wm[dm*128+hi, hc*128+hin]
    wm_p = (
        np.ascontiguousarray(
            np.asarray(Wm, np.float32).reshape(DMC, P, HC, P).transpose(1, 2, 0, 3)
        )
        .reshape(P, DM * HC * P // P)
        .astype(_np_in_dt())
    )
    # Wout rows are n*H + hc*128 + p; kernel chunk id c = hc*N + n (hc-major)
    wo_p = np.ascontiguousarray(
        np.asarray(Wout, np.float32).reshape(N, HC, P, 4).transpose(2, 1, 0, 3)
    ).reshape(P, NCHUNK * 4)
    if R_DT == bf16:
        import ml_dtypes

        wo_p = wo_p.astype(ml_dtypes.bfloat16)
    in_maps = []
    for b in range(B):
        qT_p = _pack_partition_major(
            np.ascontiguousarray(np.asarray(query[b], np.float32).T), DQC
        )
        mT_p = _pack_partition_major(
            np.ascontiguousarray(np.asarray(memory[b], np.float32).T), DMC
        )
        m = {"qT": qT_p, "mT": mT_p, "wq": wq_p, "wm": wm_p, "wo": wo_p}
        if COL_TILE > 1:
            m["sel"] = _sel_array()
        in_maps.append(m)
    return in_maps


def bench(inputs, iters=20):
    """Time repeated executions of the compiled kernel with inputs resident
    on device. Returns a list of per-call wall seconds."""
    import time

    import jax
    from jax.sharding import Mesh, PartitionSpec
    from jax.experimental.shard_map import shard_map

    from concourse import bass2jax, mybir as _mybir

    if "nc" not in _CACHE:
        _CACHE["nc"] = _build()
    nc = _CACHE["nc"]
    in_maps = _make_in_maps(**inputs)

    bass2jax.install_neuronx_cc_hook()
    partition_name = nc.partition_id_tensor.name if nc.partition_id_tensor else None
    in_names, out_names, out_avals, zero_outs = [], [], [], []
    for alloc in nc.m.functions[0].allocations:
        if not isinstance(alloc, _mybir.MemoryLocationSet):
            continue
        name = alloc.memorylocations[0].name
        if alloc.kind == "ExternalInput":
            if name != partition_name:
                in_names.append(name)
        elif alloc.kind == "ExternalOutput":
            shape = tuple(alloc.tensor_shape)
            dtype = _mybir.dt.np(alloc.dtype)
            out_names.append(name)
            out_avals.append(jax.core.ShapedArray(shape, dtype))
            zero_outs.append(np.zeros(shape, dtype))
    n_params = len(in_names)
    n_outs = len(out_avals)
    all_in_names = list(in_names) + list(out_names)
    if partition_name is not None:
        all_in_names.append(partition_name)

    def _body(*args):
        operands = list(args)
        if partition_name is not None:
            operands.append(bass2jax.partition_id_tensor())
        outs = bass2jax._bass_exec_p.bind(
            *operands,
            out_avals=tuple(out_avals),
            in_names=tuple(all_in_names),
            out_names=tuple(out_names),
            lowering_input_output_aliases=(),
            sim_require_finite=True,
            sim_require_nnan=True,
            nc=nc,
        )
        return tuple(outs)

    devices = jax.devices()[:B]
    mesh = Mesh(np.asarray(devices), ("core",))
    in_specs = (PartitionSpec("core"),) * (n_params + n_outs)
    out_specs = (PartitionSpec("core"),) * n_outs
    sharded = jax.jit(
        shard_map(
            _body, mesh=mesh, in_specs=in_specs, out_specs=out_specs, check_rep=False
        ),
        donate_argnums=tuple(range(n_params, n_params + n_outs)),
        keep_unused=True,
    )
    concat_in = [
        np.concatenate([np.asarray(in_maps[c][nm]) for c in range(B)], axis=0)
        for nm in in_names
    ]
    dev_in = [jax.device_put(a) for a in concat_in]

    def zeros():
        return [np.zeros((B * z.shape[0], *z.shape[1:]), z.dtype) for z in zero_outs]

    # warmup (compile)
    out = sharded(*dev_in, *zeros())
    jax.block_until_ready(out)

    times = []
    for _ in range(iters):
        t0 = time.perf_counter()
        out = sharded(*dev_in, *zeros())
        jax.block_until_ready(out)
        times.append(time.perf_counter() - t0)
    return times
